# revision 30
# baseline (speedup 1.0000x reference)
"""Trainium2 Bass kernel for AutoregressiveConvLSTM log-prob.

Strategy
--------
Data-parallel over batch: 64 images -> 8 NeuronCores, 8 images each.

Per-core layout: each (image-batch, channel) "plane" is an SBUF tile
[H=128 partitions, 1042 free] where image b occupies flat columns
130*b+1 .. 130*b+128 and the surrounding columns are zero pads.

All 3x3 convs run on the TensorEngine as banded matmuls:
  out[h_out, col] = sum_h_in Band[h_in, h_out] * plane[h_in, col+dx]
where Band is a [128,128] tri-diagonal matrix holding the three dy taps
(built on the host from the conv weights) and the dx in {-1,0,1} shift
is a free-dim AP offset into the zero pads.  Contributions over
(cin, dx) accumulate in PSUM.  Matmuls use float32r (full fp32 data,
fast PE mode).  LSTM pointwise math runs on ScalarE/VectorE in fp32.

The per-pixel log-prob terms are reduced over W on VectorE into a
[128 (=H), 8 (=image)] accumulator, and over H at the end with a single
ones-vector matmul.  Output per core: [8] -> host concatenates to [64].
"""

import numpy as np

B_FULL, C, H, W, F = 64, 16, 128, 128, 2
NCORES = 8
BL = B_FULL // NCORES            # images per core
WB = W + 2                       # per-image block width incl. pads
FREE = BL * WB + 2               # flat free size (+2 spare zero cols)
HALF_LOG_2PI = 0.9189385332046727

# chunks: (b0, n_imgs, c0, ilo)  — psum columns [c0, c0+n*WB), image b
# starts at local column WB*(b-b0)+ilo, interior slice [ilo, ilo+128)
CHUNKS = [(0, 3, 1, 0), (3, 3, 3 * WB, 1), (6, 2, 6 * WB, 1)]

N_STEP_BANDS = 3 + 72 + 12 + 12          # u, gates, head1, head2
N_ONETIME_BANDS = 96 + 12 + 12           # cond1, cond2, partial1
NB = N_ONETIME_BANDS + N_STEP_BANDS


def _band(w3):
    """[128,128] B[h_in,h_out] = w3[h_in-h_out+1] (tri-diagonal)."""
    b = np.zeros((H, H), np.float32)
    for dy in (-1, 0, 1):
        ar = np.arange(max(0, -dy), H - max(0, dy))
        b[ar + dy, ar] = w3[dy + 1]
    return b


def _build_bands(Wci, Wc1, Wc2, Wo1, Wo2, Wih, Whh):
    bands = np.zeros((NB, H, H), np.float32)
    i = 0
    # one-time: cond1 (16->2, ci-major for group streaming), cond2,
    # partial1 (cond_f part of Wo1)
    for ci in range(16):
        for co in range(2):
            for dx in range(3):
                bands[i] = _band(Wc1[:, dx, ci, co]); i += 1
    for co in range(2):
        for ci in range(2):
            for dx in range(3):
                bands[i] = _band(Wc2[:, dx, ci, co]); i += 1
    for co in range(2):
        for ci in range(2):
            for dx in range(3):
                bands[i] = _band(Wo1[:, dx, 2 + ci, co]); i += 1
    assert i == N_ONETIME_BANDS
    # step bands: u conv (1->1)
    for dx in range(3):
        bands[i] = _band(Wci[:, dx, 0, 0]); i += 1
    # gates: src 0,1 = h planes (Whh), src 2 = u plane (Wih)
    for co in range(8):
        for src in range(3):
            for dx in range(3):
                w3 = Whh[:, dx, src, co] if src < 2 else Wih[:, dx, 0, co]
                bands[i] = _band(w3); i += 1
    # head1 (h part of Wo1), head2 (Wo2)
    for co in range(2):
        for ci in range(2):
            for dx in range(3):
                bands[i] = _band(Wo1[:, dx, ci, co]); i += 1
    for co in range(2):
        for ci in range(2):
            for dx in range(3):
                bands[i] = _band(Wo2[:, dx, ci, co]); i += 1
    assert i == NB
    return bands


def _zero_pair(a, b):
    return np.concatenate([a, b], axis=1)


def _build_pair_bands(Wih, Whh, Wo1, Wo2):
    """52 fp8 DoubleRow stationaries [128, 2*128]: per gate co 5 pairs
    (h0 dx-/+, h1 dx-/+, u dx-/+, (h0,h1) dx0, (u,zero) dx0); per head1 co
    3 pairs from Wo1[:, :, :2]; per head2 co 3 pairs from Wo2."""
    import ml_dtypes
    Z = np.zeros((H, H), np.float32)
    out = []
    for co in range(8):
        out.append(_zero_pair(_band(Whh[:, 0, 0, co]), _band(Whh[:, 2, 0, co])))
        out.append(_zero_pair(_band(Whh[:, 0, 1, co]), _band(Whh[:, 2, 1, co])))
        out.append(_zero_pair(_band(Wih[:, 0, 0, co]), _band(Wih[:, 2, 0, co])))
        out.append(_zero_pair(_band(Whh[:, 1, 0, co]), _band(Whh[:, 1, 1, co])))
        out.append(_zero_pair(_band(Wih[:, 1, 0, co]), Z))
    for Wx in (Wo1, Wo2):
        for co in range(2):
            out.append(_zero_pair(_band(Wx[:, 0, 0, co]), _band(Wx[:, 2, 0, co])))
            out.append(_zero_pair(_band(Wx[:, 0, 1, co]), _band(Wx[:, 2, 1, co])))
            out.append(_zero_pair(_band(Wx[:, 1, 0, co]), _band(Wx[:, 1, 1, co])))
    Iband = np.eye(H, dtype=np.float32)
    out.append(_zero_pair(Iband, Z))
    out.append(_zero_pair(Iband, Z))
    return np.stack(out).astype(ml_dtypes.float8_e4m3)


def _build_program(bci, bc1, bc2, bo1, bo2, bih):
    import concourse.bacc as bacc
    import concourse.mybir as mybir
    import concourse.tile as tile

    f32 = mybir.dt.float32
    bf16x = mybir.dt.bfloat16
    MM = mybir.dt.float32r
    AF = mybir.ActivationFunctionType
    OP = mybir.AluOpType
    AX = mybir.AxisListType

    nc = bacc.Bacc("TRN2", target_bir_lowering=False, debug=False)
    xd = nc.dram_tensor("x", [BL, C, H, W], bf16x, kind="ExternalInput")
    cd = nc.dram_tensor("cond", [BL, C, H, W], MM, kind="ExternalInput")
    bd = nc.dram_tensor("bands", [NB, H, H], MM, kind="ExternalInput")
    od = nc.dram_tensor("out", [BL, 1], f32, kind="ExternalOutput")
    F8 = mybir.dt.float8e4
    pbd = nc.dram_tensor("pbands", [54, H, 256], F8, kind="ExternalInput")
    ubd = nc.dram_tensor("ubands", [3, H, H], bf16x, kind="ExternalInput")
    DR = mybir.MatmulPerfMode.DoubleRow

    def i3(ap_flat, b0, n, lo):
        # [128, n, 128] interior view of a [128, >=1040] flat AP
        return ap_flat[:, : BL * WB].rearrange(
            "p (b w) -> p b w", w=WB)[:, b0:b0 + n, lo:lo + 128]

    with tile.TileContext(nc) as tc:
        import contextlib
        ctx = contextlib.ExitStack()
        with ctx:
            state = ctx.enter_context(tc.tile_pool(name="state", bufs=1))
            sbands = ctx.enter_context(tc.tile_pool(name="sbands", bufs=1))
            stream = ctx.enter_context(tc.tile_pool(name="stream", bufs=2))
            ctmp = ctx.enter_context(tc.tile_pool(name="ctmp", bufs=2))
            tmp = ctx.enter_context(tc.tile_pool(name="tmp", bufs=5))
            psum = ctx.enter_context(
                tc.tile_pool(name="psum", bufs=8, space="PSUM"))

            # step bands: only the u-conv (3) stay fp32r; gates/heads use
            # fp8 DoubleRow pair-bands (two stationaries per pass).
            sb = sbands.tile([H, 3 * H], MM, tag="sb", name="sb")
            nc.sync.dma_start(
                sb[:, :], bd[N_ONETIME_BANDS:N_ONETIME_BANDS + 3].rearrange(
                    "n p m -> p n m"))

            def band_st(i):
                return sb[:, i * H:(i + 1) * H].bitcast(MM)

            ub = sbands.tile([H, 3 * H], bf16x, tag="ub", name="ub")
            nc.sync.dma_start(ub[:, :], ubd[:].rearrange("n p m -> p n m"))

            def band_ub(i):
                return ub[:, i * H:(i + 1) * H]

            pb = sbands.tile([H, 54 * 256], F8, tag="pb", name="pb")
            for k in range(3):
                nc.sync.dma_start(pb[:, k * 18 * 256:(k + 1) * 18 * 256],
                                  pbd[k * 18:(k + 1) * 18].rearrange(
                                      "n p m -> p n m"))

            def pband(i):
                return pb[:, i * 256:(i + 1) * 256].rearrange(
                    "p (two m) -> p two m", two=2)

            from concourse.ap import AP as _AP

            WTOT = 20 * FREE + WB * 3 + 2
            P_H0, P_H1, P_Z = 0, FREE, 17 * FREE

            def P_P1(co):
                return (18 + co) * FREE

            def P_U(t):
                return (1 + t) * FREE
            smeg = state.tile([H, WTOT], F8, tag="smeg", name="smeg")
            nc.gpsimd.memset(smeg[:], 0.0)
            s2meg = state.tile([H, 2 * FREE], F8, tag="s2meg", name="s2meg")
            nc.gpsimd.memset(s2meg[:], 0.0)

            def mslice(base):
                return smeg[:, base:base + FREE]

            def pair_mv(meg, off_a, off_b, n_cols):
                a0 = meg[:]
                return _AP(a0.tensor, a0.offset + off_a,
                           [list(a0.ap[0]), [off_b - off_a, 2], [1, n_cols]])

            # persistent planes
            def plane(tag, dt=MM, memset=True):
                t = state.tile([H, FREE], dt, tag=tag)
                if memset:
                    if dt == MM:
                        nc.gpsimd.memset(t[:].bitcast(f32), 0.0)
                    else:
                        nc.gpsimd.memset(t[:], 0.0)
                return t

            c_pl = [plane("c0", bf16x), plane("c1", bf16x)]
            p1_pl = [plane("p1a", f32), plane("p1b", f32)]
            lp = state.tile([H, BL], f32, tag="lp", name="lp")
            nc.vector.memset(lp[:], 0.0)
            ones = state.tile([H, 1], f32, tag="ones", name="ones")
            nc.vector.memset(ones[:], 1.0)
            # bias columns: 0-7 bih, 8-9 bc1, 10-11 bc2, 12-13 bo1, 14 bci,
            # 15 = -bo2[1], 16 = final output bias
            cst = -16.0 * 128.0 * 128.0 * HALF_LOG_2PI
            bias_vals = (list(bih) + list(bc1) + list(bc2) + list(bo1)
                         + [float(bci[0]), -float(bo2[1]), cst])
            bias_t = state.tile([H, 17], f32, tag="bias", name="bias")
            for j, v in enumerate(bias_vals):
                nc.vector.memset(bias_t[:, j:j + 1], float(v))

            def bap(j, p=H):
                return bias_t[:p, j:j + 1]

            def new_plane(pool, src_dram, ci, tag, bufs=None, dt=MM):
                t = pool.tile([H, FREE], dt, tag=tag, name=tag, bufs=bufs)
                t3 = t[:, : BL * WB].rearrange("p (b w) -> p b w", w=WB)
                bc = (lambda a: a.bitcast(f32)) if dt == MM else (lambda a: a)
                nc.gpsimd.memset(bc(t3[:, :, 0:1]), 0.0)
                nc.gpsimd.memset(bc(t3[:, :, WB - 1:WB]), 0.0)
                nc.gpsimd.memset(bc(t[:, BL * WB:]), 0.0)
                nc.sync.dma_start(
                    t3[:, :, 1:129], src_dram[:, ci].rearrange("b h w -> h b w"))
                return t

            x_planes = {}

            def get_x(ci):
                if ci not in x_planes:
                    x_planes[ci] = new_plane(stream, xd, ci, "xpl",
                                             bufs=16, dt=bf16x)
                return x_planes[ci]

            # ---------------- cond phase ----------------
            with tc.tile_pool(name="otbands", bufs=2) as otp:
                GRP = 6
                ob_cur = [None]

                def load_group(g):
                    ob = otp.tile([H, GRP * H], MM, tag="ob", name="ob")
                    nc.sync.dma_start(
                        ob[:, :], bd[g * GRP:(g + 1) * GRP].rearrange(
                            "n p m -> p n m"))
                    ob_cur[0] = ob

                def band_ot(i):
                    j = i % GRP
                    return ob_cur[0][:, j * H:(j + 1) * H].bitcast(MM)

                # cond1: 16 -> 2, tanh
                pc = {}
                for co in range(2):
                    for k, (b0, n, c0, lo) in enumerate(CHUNKS):
                        pc[(co, k)] = psum.tile([H, 3 * WB], f32, tag="ps", name="ps")
                loaded = [-1]

                def need_group(i):
                    g = i // GRP
                    if g != loaded[0]:
                        load_group(g)
                        loaded[0] = g

                for ci in range(16):
                    need_group(ci * 6)
                    cpl = new_plane(stream, cd, ci, "cpl", bufs=2)
                    cf = cpl[:].bitcast(MM)
                    for co in range(2):
                        for k, (b0, n, c0, lo) in enumerate(CHUNKS):
                            for dx in (-1, 0, 1):
                                nc.tensor.matmul(
                                    pc[(co, k)][:, :n * WB],
                                    band_ot(ci * 6 + co * 3 + (dx + 1)),
                                    cf[:, c0 + dx:c0 + dx + n * WB],
                                    start=(ci == 0 and dx == -1),
                                    stop=(ci == 15 and dx == 1))
                tc_pl = [ctmp.tile([H, FREE], MM, tag="tc", name="tc") for _ in range(2)]
                for t in tc_pl:
                    nc.vector.memset(t[:].bitcast(f32), 0.0)
                for co in range(2):
                    for k, (b0, n, c0, lo) in enumerate(CHUNKS):
                        p3 = pc[(co, k)][:, :n * WB].rearrange(
                            "p (b w) -> p b w", w=WB)[:, :, lo:lo + 128]
                        nc.scalar.activation(
                            i3(tc_pl[co][:], b0, n, 1), p3, AF.Tanh,
                            bias=bap(8 + co))

                # cond2 -> cond_f planes; then partial1 = conv(cond_f)+bo1
                cf_pl = [ctmp.tile([H, FREE], MM, tag="cf", name="cf") for _ in range(2)]
                for t in cf_pl:
                    nc.vector.memset(t[:].bitcast(f32), 0.0)
                pass
                for dst, srcs, base, bias_col, out_pl in (
                        (cf_pl, tc_pl, 96, 10, None),
                        (None, cf_pl, 108, 12, p1_pl)):
                    tgt = dst if dst is not None else out_pl
                    for co in range(2):
                        need_group(base + co * 6)
                        for k, (b0, n, c0, lo) in enumerate(CHUNKS):
                            pq = psum.tile([H, 3 * WB], f32, tag="ps", name="ps")
                            first = True
                            for ci in range(2):
                                sf = srcs[ci][:].bitcast(MM)
                                for dx in (-1, 0, 1):
                                    nc.tensor.matmul(
                                        pq[:, :n * WB],
                                        band_ot(base + co * 6 + ci * 3 + dx + 1),
                                        sf[:, c0 + dx:c0 + dx + n * WB],
                                        start=first,
                                        stop=(ci == 1 and dx == 1))
                                    first = False
                            p3 = pq[:, :n * WB].rearrange(
                                "p (b w) -> p b w", w=WB)[:, :, lo:lo + 128]
                            nc.scalar.activation(
                                i3(tgt[co][:], b0, n, 1), p3, AF.Identity,
                                bias=bap(bias_col + co))

            # ---------------- steps ----------------
            # deferred-lp stores: d = (mean - x), s = log_std + bo2[1], both
            # bf16 interior-only [128, 16 steps * 8 imgs * 128].  Keeps Exp
            # out of the step loop so the Act engine stays on the sigmoid
            # table (sigmoid/tanh/relu/identity in one set -> no reloads).
            bf16 = mybir.dt.bfloat16
            dst_d = state.tile([H, 16 * BL * 128], bf16, tag="dstd", name="dst_d")
            dst_s = state.tile([H, 16 * BL * 128], bf16, tag="dsts", name="dst_s")

            def lp_tail(pq0, pq1, xt, st, b0, n, c0, lo):
                NN = n * WB
                p0 = pq0[:, :NN].rearrange("p (b w) -> p b w", w=WB)[:, :, lo:lo + 128]
                p1 = pq1[:, :NN].rearrange("p (b w) -> p b w", w=WB)[:, :, lo:lo + 128]
                x3 = xt[:, :BL * WB].rearrange(
                    "p (b w) -> p b w", w=WB)[:, b0:b0 + n, 1:129]
                d3 = dst_d[:, st * BL * 128:(st + 1) * BL * 128].rearrange(
                    "p (b w) -> p b w", w=128)[:, b0:b0 + n, :]
                s3 = dst_s[:, st * BL * 128:(st + 1) * BL * 128].rearrange(
                    "p (b w) -> p b w", w=128)[:, b0:b0 + n, :]
                nc.vector.scalar_tensor_tensor(
                    d3, p0, float(bo2[0]), x3, OP.add, OP.subtract)
                nc.scalar.activation(s3, p1, AF.Identity, scale=-2.0,
                                     bias=-2.0 * float(bo2[1]))

            def head_pairs(q, NN, base_band, meg, pa, pb_, c0):
                prs = [(pa + c0 - 1, pa + c0 + 1), (pb_ + c0 - 1, pb_ + c0 + 1),
                       (pa + c0, pb_ + c0)]
                for j, (oa, ob) in enumerate(prs):
                    nc.tensor.matmul(q[:, :NN], pband(base_band + j),
                                     pair_mv(meg, oa, ob, NN),
                                     start=(j == 0), stop=(j == 2),
                                     perf_mode=DR)

            def head2_and_lp(xt_pl, st, b0, n, c0, lo):
                NN = n * WB
                pq = []
                for co in range(2):
                    q = psum.tile([H, 3 * WB], f32, tag="ps", name="ps")
                    head_pairs(q, NN, 46 + co * 3, s2meg, 0, FREE, c0)
                    pq.append(q)
                lp_tail(pq[0], pq[1], xt_pl, st, b0, n, c0, lo)

            # p1 as fp8 planes (for the identity-band head1 psum-init pass)
            for co in range(2):
                nc.scalar.activation(
                    i3(smeg[:, P_P1(co):P_P1(co) + FREE], 0, BL, 1),
                    i3(p1_pl[co][:], 0, BL, 1), AF.Identity)

            # u planes for all steps precomputed off the h-recurrence:
            # u_t depends only on x_{t-1}, so these fill PE/Act idle slots
            # and the per-step gate matmuls never wait on a shared u slot.
            for ust in range(1, 16):
                xp = get_x(ust - 1)
                for (b0, n, c0, lo) in CHUNKS:
                    NN = n * WB
                    pu = psum.tile([H, 3 * WB], f32, tag="ps", name="ps")
                    for dx in (-1, 0, 1):
                        nc.tensor.matmul(pu[:, :NN], band_ub(dx + 1),
                                         xp[:, c0 + dx:c0 + dx + NN],
                                         start=(dx == -1), stop=(dx == 1))
                    p3 = pu[:, :NN].rearrange(
                        "p (b w) -> p b w", w=WB)[:, :, lo:lo + 128]
                    nc.scalar.activation(
                        i3(smeg[:, P_U(ust):P_U(ust) + FREE], b0, n, 1), p3,
                        AF.Identity, bias=bap(14))

            # step 0: feat = 0 -> r = relu(partial1)
            x0 = get_x(0)
            for (b0, n, c0, lo) in CHUNKS:
                for co in range(2):
                    nc.scalar.activation(
                        i3(s2meg[:, co * FREE:co * FREE + FREE], b0, n, 1),
                        i3(p1_pl[co][:], b0, n, 1), AF.Relu)
                head2_and_lp(x0, 0, b0, n, c0, lo)

            for st in range(1, 16):
                xt = get_x(st)
                for (b0, n, c0, lo) in CHUNKS:
                    NN = n * WB
                    # gates: 5 fp8 DoubleRow pair-passes per channel
                    pg = [None] * 8
                    for co in (0, 2, 4, 6, 1, 3, 5, 7):
                        g = psum.tile([H, 3 * WB], f32, tag="ps", name="ps")
                        pu_ = P_U(st)
                        prs = [(P_H0 + c0 - 1, P_H0 + c0 + 1),
                               (P_H1 + c0 - 1, P_H1 + c0 + 1),
                               (pu_ + c0 - 1, pu_ + c0 + 1),
                               (P_H0 + c0, P_H1 + c0),
                               (pu_ + c0, P_Z)]
                        for j, (oa, ob) in enumerate(prs):
                            nc.tensor.matmul(g[:, :NN], pband(co * 5 + j),
                                             pair_mv(smeg, oa, ob, NN),
                                             start=(j == 0), stop=(j == 4),
                                             perf_mode=DR)
                        pg[co] = g
                    # LSTM pointwise (i,f,g,o = pg[0:2],[2:4],[4:6],[6:8])
                    for f in range(2):
                        ti = tmp.tile([H, NN], bf16x, tag="tb", name="ti")
                        nc.scalar.activation(ti[:], pg[f][:, :NN], AF.Sigmoid,
                                             bias=bap(f))
                        tg = tmp.tile([H, NN], bf16x, tag="tb", name="tg")
                        nc.scalar.activation(tg[:], pg[4 + f][:, :NN], AF.Tanh,
                                             bias=bap(4 + f))
                        tf = tmp.tile([H, NN], bf16x, tag="tb", name="tf")
                        nc.scalar.activation(tf[:], pg[2 + f][:, :NN],
                                             AF.Sigmoid, bias=bap(2 + f))
                        to = tmp.tile([H, NN], bf16x, tag="tb", name="to")
                        nc.scalar.activation(to[:], pg[6 + f][:, :NN],
                                             AF.Sigmoid, bias=bap(6 + f))
                        tig = tmp.tile([H, NN], bf16x, tag="tb", name="tig")
                        nc.vector.scalar_tensor_tensor(
                            tig[:], ti[:], 1.0, tg[:], OP.mult, OP.mult)
                        csl = c_pl[f][:, c0:c0 + NN]
                        nc.vector.scalar_tensor_tensor(
                            csl, tf[:], 1.0, csl, OP.mult, OP.mult)
                        nc.vector.scalar_tensor_tensor(
                            csl, csl, 1.0, tig[:], OP.mult, OP.add)
                        tc_ = tmp.tile([H, NN], bf16x, tag="tb", name="tc_")
                        nc.scalar.activation(tc_[:], csl, AF.Tanh)
                        to3 = to[:].rearrange(
                            "p (b w) -> p b w", w=WB)[:, :, lo:lo + 128]
                        tc3 = tc_[:].rearrange(
                            "p (b w) -> p b w", w=WB)[:, :, lo:lo + 128]
                        nc.vector.tensor_tensor(
                            i3(mslice(f * FREE), b0, n, 1), to3, tc3, OP.mult)
                    # head1: r = relu(conv(h,Wo1[:, :, :2]) + partial1)
                    for co in range(2):
                        ph = psum.tile([H, 3 * WB], f32, tag="ps", name="ps")
                        prs = [(P_H0 + c0 - 1, P_H0 + c0 + 1),
                               (P_H1 + c0 - 1, P_H1 + c0 + 1),
                               (P_H0 + c0, P_H1 + c0),
                               (P_P1(co) + c0, P_Z)]
                        bnds = [40 + co * 3, 40 + co * 3 + 1,
                                40 + co * 3 + 2, 52 + co]
                        for j, (oa, ob) in enumerate(prs):
                            nc.tensor.matmul(ph[:, :NN], pband(bnds[j]),
                                             pair_mv(smeg, oa, ob, NN),
                                             start=(j == 0), stop=(j == 3),
                                             perf_mode=DR)
                        hp3 = ph[:, :NN].rearrange(
                            "p (b w) -> p b w", w=WB)[:, :, lo:lo + 128]
                        nc.scalar.activation(
                            i3(s2meg[:, co * FREE:co * FREE + FREE], b0, n, 1),
                            hp3, AF.Relu)
                    head2_and_lp(xt, st, b0, n, c0, lo)

            # ---------------- deferred lp phase (poly-exp on DVE) ----------------
            # stored y = -2*(log_std+bo2[1]); lp += sum_w(0.5*d^2*e^y - y/2)
            # e^y via degree-4 polynomial (|y| < 0.75, rel err ~5e-4), all on
            # DVE so no activation-table switches; the scheduler freely
            # overlaps these with the step-loop matmuls.
            A0, A1, A2, A3, A4 = (1.0000054, 0.99936129, 0.49979731,
                                  0.17194260, 0.04274371)
            IW = BL * 128
            for st in range(16):
                sl = slice(st * IW, (st + 1) * IW)
                yv = dst_s[:, sl]
                t1 = tmp.tile([H, IW], bf16, tag="tq", name="t1", bufs=4)
                nc.vector.tensor_scalar(t1[:], yv, A4, None, OP.mult)
                t2 = tmp.tile([H, IW], bf16, tag="tq", name="t2", bufs=4)
                nc.vector.scalar_tensor_tensor(t2[:], t1[:], A3, yv,
                                               OP.add, OP.mult)
                t3 = tmp.tile([H, IW], bf16, tag="tq", name="t3", bufs=4)
                nc.vector.scalar_tensor_tensor(t3[:], t2[:], A2, yv,
                                               OP.add, OP.mult)
                t4 = tmp.tile([H, IW], bf16, tag="tq", name="t4", bufs=4)
                nc.vector.scalar_tensor_tensor(t4[:], t3[:], A1, yv,
                                               OP.add, OP.mult)
                q = tmp.tile([H, IW], bf16, tag="tq", name="q", bufs=4)
                nc.vector.scalar_tensor_tensor(
                    q[:], dst_d[:, sl], 0.5, dst_d[:, sl], OP.mult, OP.mult)
                t5 = tmp.tile([H, IW], bf16, tag="tq", name="t5", bufs=4)
                nc.vector.scalar_tensor_tensor(t5[:], t4[:], A0, q[:],
                                               OP.add, OP.mult)
                t6 = tmp.tile([H, IW], bf16, tag="tq", name="t6", bufs=4)
                nc.vector.scalar_tensor_tensor(t6[:], yv, -0.5, t5[:],
                                               OP.mult, OP.add)
                red = tmp.tile([H, BL], f32, tag="tr", name="red", bufs=3)
                nc.vector.reduce_sum(
                    red[:], t6[:].rearrange("p (b w) -> p b w", w=128), AX.X)
                nc.vector.tensor_tensor(lp[:], lp[:], red[:], OP.add)

            # final: out = -(sum_p lp) - 16*128*128*HALF_LOG_2PI
            po = psum.tile([BL, 1], f32, tag="ps", name="ps")
            nc.tensor.matmul(po[:], lp[:], ones[:], start=True, stop=True)
            osb = state.tile([BL, 1], f32, tag="osb", name="osb")
            nc.scalar.activation(osb[:], po[:], AF.Identity,
                                 scale=-1.0, bias=bap(16, BL))
            nc.sync.dma_start(od[:], osb[:])
    nc.compile()
    return nc


def kernel(**inputs):
    import ml_dtypes
    x = np.ascontiguousarray(inputs["x"], np.float32).astype(ml_dtypes.bfloat16)
    cond = np.ascontiguousarray(inputs["cond"], np.float32)
    bands = _build_bands(
        np.asarray(inputs["Wci"], np.float32),
        np.asarray(inputs["Wc1"], np.float32),
        np.asarray(inputs["Wc2"], np.float32),
        np.asarray(inputs["Wo1"], np.float32),
        np.asarray(inputs["Wo2"], np.float32),
        np.asarray(inputs["Wih"], np.float32),
        np.asarray(inputs["Whh"], np.float32))
    ubands = _build_bands(
        np.asarray(inputs["Wci"], np.float32),
        np.asarray(inputs["Wc1"], np.float32),
        np.asarray(inputs["Wc2"], np.float32),
        np.asarray(inputs["Wo1"], np.float32),
        np.asarray(inputs["Wo2"], np.float32),
        np.asarray(inputs["Wih"], np.float32),
        np.asarray(inputs["Whh"], np.float32))[
        N_ONETIME_BANDS:N_ONETIME_BANDS + 3].astype(ml_dtypes.bfloat16)
    pbands = _build_pair_bands(
        np.asarray(inputs["Wih"], np.float32),
        np.asarray(inputs["Whh"], np.float32),
        np.asarray(inputs["Wo1"], np.float32),
        np.asarray(inputs["Wo2"], np.float32))
    nc = _build_program(
        np.asarray(inputs["bci"], np.float32),
        np.asarray(inputs["bc1"], np.float32),
        np.asarray(inputs["bc2"], np.float32),
        np.asarray(inputs["bo1"], np.float32),
        np.asarray(inputs["bo2"], np.float32),
        np.asarray(inputs["bih"], np.float32))
    from concourse.bass_utils import run_bass_kernel_spmd
    in_maps = [
        {"x": x[i * BL:(i + 1) * BL], "cond": cond[i * BL:(i + 1) * BL],
         "bands": bands, "pbands": pbands, "ubands": ubands}
        for i in range(NCORES)
    ]
    res = run_bass_kernel_spmd(nc, in_maps, list(range(NCORES)))
    out = np.concatenate(
        [res.results[i]["out"].reshape(BL) for i in range(NCORES)])
    return out.astype(np.float32)


if __name__ == "__main__":
    # smoke test with tiny random weights
    rng = np.random.default_rng(0)
    ins = {
        "x": rng.standard_normal((64, 16, 128, 128), np.float32),
        "cond": rng.standard_normal((64, 16, 128, 128), np.float32),
        "Wci": rng.standard_normal((3, 3, 1, 1), np.float32) * 0.1,
        "bci": np.zeros(1, np.float32),
        "Wc1": rng.standard_normal((3, 3, 16, 2), np.float32) * 0.1,
        "bc1": np.zeros(2, np.float32),
        "Wc2": rng.standard_normal((3, 3, 2, 2), np.float32) * 0.1,
        "bc2": np.zeros(2, np.float32),
        "Wo1": rng.standard_normal((3, 3, 4, 2), np.float32) * 0.1,
        "bo1": np.zeros(2, np.float32),
        "Wo2": rng.standard_normal((3, 3, 2, 2), np.float32) * 0.1,
        "bo2": np.zeros(2, np.float32),
        "Wih": rng.standard_normal((3, 3, 1, 8), np.float32) * 0.1,
        "bih": np.zeros(8, np.float32),
        "Whh": rng.standard_normal((3, 3, 2, 8), np.float32) * 0.1,
    }
    print(kernel(**ins)[:8])



# revision 31
# speedup vs baseline: 1.0138x; 1.0138x over previous
"""Trainium2 Bass kernel for AutoregressiveConvLSTM log-prob.

Strategy
--------
Data-parallel over batch: 64 images -> 8 NeuronCores, 8 images each.

Per-core layout: each (image-batch, channel) "plane" is an SBUF tile
[H=128 partitions, 1042 free] where image b occupies flat columns
130*b+1 .. 130*b+128 and the surrounding columns are zero pads.

All 3x3 convs run on the TensorEngine as banded matmuls:
  out[h_out, col] = sum_h_in Band[h_in, h_out] * plane[h_in, col+dx]
where Band is a [128,128] tri-diagonal matrix holding the three dy taps
(built on the host from the conv weights) and the dx in {-1,0,1} shift
is a free-dim AP offset into the zero pads.  Contributions over
(cin, dx) accumulate in PSUM.  Matmuls use float32r (full fp32 data,
fast PE mode).  LSTM pointwise math runs on ScalarE/VectorE in fp32.

The per-pixel log-prob terms are reduced over W on VectorE into a
[128 (=H), 8 (=image)] accumulator, and over H at the end with a single
ones-vector matmul.  Output per core: [8] -> host concatenates to [64].
"""

import numpy as np

B_FULL, C, H, W, F = 64, 16, 128, 128, 2
NCORES = 8
BL = B_FULL // NCORES            # images per core
WB = W + 2                       # per-image block width incl. pads
FREE = BL * WB + 2               # flat free size (+2 spare zero cols)
HALF_LOG_2PI = 0.9189385332046727

# chunks: (b0, n_imgs, c0, ilo)  — psum columns [c0, c0+n*WB), image b
# starts at local column WB*(b-b0)+ilo, interior slice [ilo, ilo+128)
CHUNKS = [(0, 3, 1, 0), (3, 3, 3 * WB, 1), (6, 2, 6 * WB, 1)]

N_STEP_BANDS = 3 + 72 + 12 + 12          # u, gates, head1, head2
N_ONETIME_BANDS = 96 + 12 + 12           # cond1, cond2, partial1
NB = N_ONETIME_BANDS + N_STEP_BANDS


def _band(w3):
    """[128,128] B[h_in,h_out] = w3[h_in-h_out+1] (tri-diagonal)."""
    b = np.zeros((H, H), np.float32)
    for dy in (-1, 0, 1):
        ar = np.arange(max(0, -dy), H - max(0, dy))
        b[ar + dy, ar] = w3[dy + 1]
    return b


def _build_bands(Wci, Wc1, Wc2, Wo1, Wo2, Wih, Whh):
    bands = np.zeros((NB, H, H), np.float32)
    i = 0
    # one-time: cond1 (16->2, ci-major for group streaming), cond2,
    # partial1 (cond_f part of Wo1)
    for ci in range(16):
        for co in range(2):
            for dx in range(3):
                bands[i] = _band(Wc1[:, dx, ci, co]); i += 1
    for co in range(2):
        for ci in range(2):
            for dx in range(3):
                bands[i] = _band(Wc2[:, dx, ci, co]); i += 1
    for co in range(2):
        for ci in range(2):
            for dx in range(3):
                bands[i] = _band(Wo1[:, dx, 2 + ci, co]); i += 1
    assert i == N_ONETIME_BANDS
    # step bands: u conv (1->1)
    for dx in range(3):
        bands[i] = _band(Wci[:, dx, 0, 0]); i += 1
    # gates: src 0,1 = h planes (Whh), src 2 = u plane (Wih)
    for co in range(8):
        for src in range(3):
            for dx in range(3):
                w3 = Whh[:, dx, src, co] if src < 2 else Wih[:, dx, 0, co]
                bands[i] = _band(w3); i += 1
    # head1 (h part of Wo1), head2 (Wo2)
    for co in range(2):
        for ci in range(2):
            for dx in range(3):
                bands[i] = _band(Wo1[:, dx, ci, co]); i += 1
    for co in range(2):
        for ci in range(2):
            for dx in range(3):
                bands[i] = _band(Wo2[:, dx, ci, co]); i += 1
    assert i == NB
    return bands


def _zero_pair(a, b):
    return np.concatenate([a, b], axis=1)


def _build_pair_bands(Wih, Whh, Wo1, Wo2):
    """52 fp8 DoubleRow stationaries [128, 2*128]: per gate co 5 pairs
    (h0 dx-/+, h1 dx-/+, u dx-/+, (h0,h1) dx0, (u,zero) dx0); per head1 co
    3 pairs from Wo1[:, :, :2]; per head2 co 3 pairs from Wo2."""
    import ml_dtypes
    Z = np.zeros((H, H), np.float32)
    out = []
    for co in range(8):
        out.append(_zero_pair(_band(Whh[:, 0, 0, co]), _band(Whh[:, 2, 0, co])))
        out.append(_zero_pair(_band(Whh[:, 0, 1, co]), _band(Whh[:, 2, 1, co])))
        out.append(_zero_pair(_band(Wih[:, 0, 0, co]), _band(Wih[:, 2, 0, co])))
        out.append(_zero_pair(_band(Whh[:, 1, 0, co]), _band(Whh[:, 1, 1, co])))
        out.append(_zero_pair(_band(Wih[:, 1, 0, co]), Z))
    for Wx in (Wo1, Wo2):
        for co in range(2):
            out.append(_zero_pair(_band(Wx[:, 0, 0, co]), _band(Wx[:, 2, 0, co])))
            out.append(_zero_pair(_band(Wx[:, 0, 1, co]), _band(Wx[:, 2, 1, co])))
            out.append(_zero_pair(_band(Wx[:, 1, 0, co]), _band(Wx[:, 1, 1, co])))
    Iband = np.eye(H, dtype=np.float32)
    out.append(_zero_pair(Iband, Z))
    out.append(_zero_pair(Iband, Z))
    return np.stack(out).astype(ml_dtypes.float8_e4m3)


def _build_program(bci, bc1, bc2, bo1, bo2, bih):
    import concourse.bacc as bacc
    import concourse.mybir as mybir
    import concourse.tile as tile

    f32 = mybir.dt.float32
    bf16x = mybir.dt.bfloat16
    MM = mybir.dt.float32r
    AF = mybir.ActivationFunctionType
    OP = mybir.AluOpType
    AX = mybir.AxisListType

    nc = bacc.Bacc("TRN2", target_bir_lowering=False, debug=False)
    xd = nc.dram_tensor("x", [BL, C, H, W], bf16x, kind="ExternalInput")
    cd = nc.dram_tensor("cond", [BL, C, H, W], MM, kind="ExternalInput")
    bd = nc.dram_tensor("bands", [NB, H, H], MM, kind="ExternalInput")
    od = nc.dram_tensor("out", [BL, 1], f32, kind="ExternalOutput")
    F8 = mybir.dt.float8e4
    pbd = nc.dram_tensor("pbands", [54, H, 256], F8, kind="ExternalInput")
    ubd = nc.dram_tensor("ubands", [3, H, H], bf16x, kind="ExternalInput")
    DR = mybir.MatmulPerfMode.DoubleRow

    def i3(ap_flat, b0, n, lo):
        # [128, n, 128] interior view of a [128, >=1040] flat AP
        return ap_flat[:, : BL * WB].rearrange(
            "p (b w) -> p b w", w=WB)[:, b0:b0 + n, lo:lo + 128]

    with tile.TileContext(nc) as tc:
        import contextlib
        ctx = contextlib.ExitStack()
        with ctx:
            state = ctx.enter_context(tc.tile_pool(name="state", bufs=1))
            sbands = ctx.enter_context(tc.tile_pool(name="sbands", bufs=1))
            stream = ctx.enter_context(tc.tile_pool(name="stream", bufs=2))
            ctmp = ctx.enter_context(tc.tile_pool(name="ctmp", bufs=2))
            tmp = ctx.enter_context(tc.tile_pool(name="tmp", bufs=5))
            psum = ctx.enter_context(
                tc.tile_pool(name="psum", bufs=8, space="PSUM"))

            # step bands: only the u-conv (3) stay fp32r; gates/heads use
            # fp8 DoubleRow pair-bands (two stationaries per pass).
            sb = sbands.tile([H, 3 * H], MM, tag="sb", name="sb")
            nc.sync.dma_start(
                sb[:, :], bd[N_ONETIME_BANDS:N_ONETIME_BANDS + 3].rearrange(
                    "n p m -> p n m"))

            def band_st(i):
                return sb[:, i * H:(i + 1) * H].bitcast(MM)

            ub = sbands.tile([H, 3 * H], bf16x, tag="ub", name="ub")
            nc.sync.dma_start(ub[:, :], ubd[:].rearrange("n p m -> p n m"))

            def band_ub(i):
                return ub[:, i * H:(i + 1) * H]

            pb = sbands.tile([H, 54 * 256], F8, tag="pb", name="pb")
            for k in range(3):
                nc.sync.dma_start(pb[:, k * 18 * 256:(k + 1) * 18 * 256],
                                  pbd[k * 18:(k + 1) * 18].rearrange(
                                      "n p m -> p n m"))

            def pband(i):
                return pb[:, i * 256:(i + 1) * 256].rearrange(
                    "p (two m) -> p two m", two=2)

            from concourse.ap import AP as _AP

            WTOT = 20 * FREE + WB * 3 + 2
            P_H0, P_H1, P_Z = 0, FREE, 17 * FREE

            def P_P1(co):
                return (18 + co) * FREE

            def P_U(t):
                return (1 + t) * FREE
            smeg = state.tile([H, WTOT], F8, tag="smeg", name="smeg")
            nc.gpsimd.memset(smeg[:], 0.0)
            s2meg = state.tile([H, 2 * FREE], F8, tag="s2meg", name="s2meg")
            nc.gpsimd.memset(s2meg[:], 0.0)

            def mslice(base):
                return smeg[:, base:base + FREE]

            def pair_mv(meg, off_a, off_b, n_cols):
                a0 = meg[:]
                return _AP(a0.tensor, a0.offset + off_a,
                           [list(a0.ap[0]), [off_b - off_a, 2], [1, n_cols]])

            # persistent planes
            def plane(tag, dt=MM, memset=True):
                t = state.tile([H, FREE], dt, tag=tag)
                if memset:
                    if dt == MM:
                        nc.gpsimd.memset(t[:].bitcast(f32), 0.0)
                    else:
                        nc.gpsimd.memset(t[:], 0.0)
                return t

            c_pl = [plane("c0", bf16x), plane("c1", bf16x)]
            p1_pl = [plane("p1a", f32), plane("p1b", f32)]
            lp = state.tile([H, BL], f32, tag="lp", name="lp")
            nc.vector.memset(lp[:], 0.0)
            ones = state.tile([H, 1], f32, tag="ones", name="ones")
            nc.vector.memset(ones[:], 1.0)
            # bias columns: 0-7 bih, 8-9 bc1, 10-11 bc2, 12-13 bo1, 14 bci,
            # 15 = -bo2[1], 16 = final output bias
            cst = -16.0 * 128.0 * 128.0 * HALF_LOG_2PI
            bias_vals = (list(bih) + list(bc1) + list(bc2) + list(bo1)
                         + [float(bci[0]), -float(bo2[1]), cst])
            bias_t = state.tile([H, 17], f32, tag="bias", name="bias")
            for j, v in enumerate(bias_vals):
                nc.vector.memset(bias_t[:, j:j + 1], float(v))

            def bap(j, p=H):
                return bias_t[:p, j:j + 1]

            def new_plane(pool, src_dram, ci, tag, bufs=None, dt=MM):
                t = pool.tile([H, FREE], dt, tag=tag, name=tag, bufs=bufs)
                t3 = t[:, : BL * WB].rearrange("p (b w) -> p b w", w=WB)
                bc = (lambda a: a.bitcast(f32)) if dt == MM else (lambda a: a)
                nc.gpsimd.memset(bc(t3[:, :, 0:1]), 0.0)
                nc.gpsimd.memset(bc(t3[:, :, WB - 1:WB]), 0.0)
                nc.gpsimd.memset(bc(t[:, BL * WB:]), 0.0)
                nc.sync.dma_start(
                    t3[:, :, 1:129], src_dram[:, ci].rearrange("b h w -> h b w"))
                return t

            x_planes = {}

            def get_x(ci):
                if ci not in x_planes:
                    x_planes[ci] = new_plane(stream, xd, ci, "xpl",
                                             bufs=16, dt=bf16x)
                return x_planes[ci]

            # ---------------- cond phase ----------------
            with tc.tile_pool(name="otbands", bufs=2) as otp:
                GRP = 6
                ob_cur = [None]

                def load_group(g):
                    ob = otp.tile([H, GRP * H], MM, tag="ob", name="ob")
                    nc.sync.dma_start(
                        ob[:, :], bd[g * GRP:(g + 1) * GRP].rearrange(
                            "n p m -> p n m"))
                    ob_cur[0] = ob

                def band_ot(i):
                    j = i % GRP
                    return ob_cur[0][:, j * H:(j + 1) * H].bitcast(MM)

                # cond1: 16 -> 2, tanh
                pc = {}
                for co in range(2):
                    for k, (b0, n, c0, lo) in enumerate(CHUNKS):
                        pc[(co, k)] = psum.tile([H, 3 * WB], f32, tag="ps", name="ps")
                loaded = [-1]

                def need_group(i):
                    g = i // GRP
                    if g != loaded[0]:
                        load_group(g)
                        loaded[0] = g

                for ci in range(16):
                    need_group(ci * 6)
                    cpl = new_plane(stream, cd, ci, "cpl", bufs=3)
                    cf = cpl[:].bitcast(MM)
                    for co in range(2):
                        for k, (b0, n, c0, lo) in enumerate(CHUNKS):
                            for dx in (-1, 0, 1):
                                nc.tensor.matmul(
                                    pc[(co, k)][:, :n * WB],
                                    band_ot(ci * 6 + co * 3 + (dx + 1)),
                                    cf[:, c0 + dx:c0 + dx + n * WB],
                                    start=(ci == 0 and dx == -1),
                                    stop=(ci == 15 and dx == 1))
                tc_pl = [ctmp.tile([H, FREE], MM, tag="tc", name="tc") for _ in range(2)]
                for t in tc_pl:
                    nc.vector.memset(t[:].bitcast(f32), 0.0)
                for co in range(2):
                    for k, (b0, n, c0, lo) in enumerate(CHUNKS):
                        p3 = pc[(co, k)][:, :n * WB].rearrange(
                            "p (b w) -> p b w", w=WB)[:, :, lo:lo + 128]
                        nc.scalar.activation(
                            i3(tc_pl[co][:], b0, n, 1), p3, AF.Tanh,
                            bias=bap(8 + co))

                # cond2 -> cond_f planes; then partial1 = conv(cond_f)+bo1
                cf_pl = [ctmp.tile([H, FREE], MM, tag="cf", name="cf") for _ in range(2)]
                for t in cf_pl:
                    nc.vector.memset(t[:].bitcast(f32), 0.0)
                pass
                for dst, srcs, base, bias_col, out_pl in (
                        (cf_pl, tc_pl, 96, 10, None),
                        (None, cf_pl, 108, 12, p1_pl)):
                    tgt = dst if dst is not None else out_pl
                    for co in range(2):
                        need_group(base + co * 6)
                        for k, (b0, n, c0, lo) in enumerate(CHUNKS):
                            pq = psum.tile([H, 3 * WB], f32, tag="ps", name="ps")
                            first = True
                            for ci in range(2):
                                sf = srcs[ci][:].bitcast(MM)
                                for dx in (-1, 0, 1):
                                    nc.tensor.matmul(
                                        pq[:, :n * WB],
                                        band_ot(base + co * 6 + ci * 3 + dx + 1),
                                        sf[:, c0 + dx:c0 + dx + n * WB],
                                        start=first,
                                        stop=(ci == 1 and dx == 1))
                                    first = False
                            p3 = pq[:, :n * WB].rearrange(
                                "p (b w) -> p b w", w=WB)[:, :, lo:lo + 128]
                            nc.scalar.activation(
                                i3(tgt[co][:], b0, n, 1), p3, AF.Identity,
                                bias=bap(bias_col + co))

            # ---------------- steps ----------------
            # deferred-lp stores: d = (mean - x), s = log_std + bo2[1], both
            # bf16 interior-only [128, 16 steps * 8 imgs * 128].  Keeps Exp
            # out of the step loop so the Act engine stays on the sigmoid
            # table (sigmoid/tanh/relu/identity in one set -> no reloads).
            bf16 = mybir.dt.bfloat16
            dst_d = state.tile([H, 16 * BL * 128], bf16, tag="dstd", name="dst_d")
            dst_s = state.tile([H, 16 * BL * 128], bf16, tag="dsts", name="dst_s")

            def lp_tail(pq0, pq1, xt, st, b0, n, c0, lo):
                NN = n * WB
                p0 = pq0[:, :NN].rearrange("p (b w) -> p b w", w=WB)[:, :, lo:lo + 128]
                p1 = pq1[:, :NN].rearrange("p (b w) -> p b w", w=WB)[:, :, lo:lo + 128]
                x3 = xt[:, :BL * WB].rearrange(
                    "p (b w) -> p b w", w=WB)[:, b0:b0 + n, 1:129]
                d3 = dst_d[:, st * BL * 128:(st + 1) * BL * 128].rearrange(
                    "p (b w) -> p b w", w=128)[:, b0:b0 + n, :]
                s3 = dst_s[:, st * BL * 128:(st + 1) * BL * 128].rearrange(
                    "p (b w) -> p b w", w=128)[:, b0:b0 + n, :]
                nc.vector.scalar_tensor_tensor(
                    d3, p0, float(bo2[0]), x3, OP.add, OP.subtract)
                nc.scalar.activation(s3, p1, AF.Identity, scale=-2.0,
                                     bias=-2.0 * float(bo2[1]))

            def head_pairs(q, NN, base_band, meg, pa, pb_, c0):
                prs = [(pa + c0 - 1, pa + c0 + 1), (pb_ + c0 - 1, pb_ + c0 + 1),
                       (pa + c0, pb_ + c0)]
                for j, (oa, ob) in enumerate(prs):
                    nc.tensor.matmul(q[:, :NN], pband(base_band + j),
                                     pair_mv(meg, oa, ob, NN),
                                     start=(j == 0), stop=(j == 2),
                                     perf_mode=DR)

            def head2_and_lp(xt_pl, st, b0, n, c0, lo):
                NN = n * WB
                pq = []
                for co in range(2):
                    q = psum.tile([H, 3 * WB], f32, tag="ps", name="ps")
                    head_pairs(q, NN, 46 + co * 3, s2meg, 0, FREE, c0)
                    pq.append(q)
                lp_tail(pq[0], pq[1], xt_pl, st, b0, n, c0, lo)

            # p1 as fp8 planes (for the identity-band head1 psum-init pass)
            for co in range(2):
                nc.scalar.activation(
                    i3(smeg[:, P_P1(co):P_P1(co) + FREE], 0, BL, 1),
                    i3(p1_pl[co][:], 0, BL, 1), AF.Identity)

            # u planes for all steps precomputed off the h-recurrence:
            # u_t depends only on x_{t-1}, so these fill PE/Act idle slots
            # and the per-step gate matmuls never wait on a shared u slot.
            for ust in range(1, 16):
                xp = get_x(ust - 1)
                for (b0, n, c0, lo) in CHUNKS:
                    NN = n * WB
                    pu = psum.tile([H, 3 * WB], f32, tag="ps", name="ps")
                    for dx in (-1, 0, 1):
                        nc.tensor.matmul(pu[:, :NN], band_ub(dx + 1),
                                         xp[:, c0 + dx:c0 + dx + NN],
                                         start=(dx == -1), stop=(dx == 1))
                    p3 = pu[:, :NN].rearrange(
                        "p (b w) -> p b w", w=WB)[:, :, lo:lo + 128]
                    nc.scalar.activation(
                        i3(smeg[:, P_U(ust):P_U(ust) + FREE], b0, n, 1), p3,
                        AF.Identity, bias=bap(14))

            # step 0: feat = 0 -> r = relu(partial1)
            x0 = get_x(0)
            for (b0, n, c0, lo) in CHUNKS:
                for co in range(2):
                    nc.scalar.activation(
                        i3(s2meg[:, co * FREE:co * FREE + FREE], b0, n, 1),
                        i3(p1_pl[co][:], b0, n, 1), AF.Relu)
                head2_and_lp(x0, 0, b0, n, c0, lo)

            for st in range(1, 16):
                xt = get_x(st)
                for (b0, n, c0, lo) in CHUNKS:
                    NN = n * WB
                    # gates: 5 fp8 DoubleRow pair-passes per channel
                    pg = [None] * 8
                    for co in (0, 2, 4, 6, 1, 3, 5, 7):
                        g = psum.tile([H, 3 * WB], f32, tag="ps", name="ps")
                        pu_ = P_U(st)
                        prs = [(P_H0 + c0 - 1, P_H0 + c0 + 1),
                               (P_H1 + c0 - 1, P_H1 + c0 + 1),
                               (pu_ + c0 - 1, pu_ + c0 + 1),
                               (P_H0 + c0, P_H1 + c0),
                               (pu_ + c0, P_Z)]
                        for j, (oa, ob) in enumerate(prs):
                            nc.tensor.matmul(g[:, :NN], pband(co * 5 + j),
                                             pair_mv(smeg, oa, ob, NN),
                                             start=(j == 0), stop=(j == 4),
                                             perf_mode=DR)
                        pg[co] = g
                    # LSTM pointwise (i,f,g,o = pg[0:2],[2:4],[4:6],[6:8])
                    for f in range(2):
                        ti = tmp.tile([H, NN], bf16x, tag="tb", name="ti")
                        nc.scalar.activation(ti[:], pg[f][:, :NN], AF.Sigmoid,
                                             bias=bap(f))
                        tg = tmp.tile([H, NN], bf16x, tag="tb", name="tg")
                        nc.scalar.activation(tg[:], pg[4 + f][:, :NN], AF.Tanh,
                                             bias=bap(4 + f))
                        tf = tmp.tile([H, NN], bf16x, tag="tb", name="tf")
                        nc.scalar.activation(tf[:], pg[2 + f][:, :NN],
                                             AF.Sigmoid, bias=bap(2 + f))
                        to = tmp.tile([H, NN], bf16x, tag="tb", name="to")
                        nc.scalar.activation(to[:], pg[6 + f][:, :NN],
                                             AF.Sigmoid, bias=bap(6 + f))
                        tig = tmp.tile([H, NN], bf16x, tag="tb", name="tig")
                        nc.vector.scalar_tensor_tensor(
                            tig[:], ti[:], 1.0, tg[:], OP.mult, OP.mult)
                        csl = c_pl[f][:, c0:c0 + NN]
                        nc.vector.scalar_tensor_tensor(
                            csl, tf[:], 1.0, csl, OP.mult, OP.mult)
                        nc.vector.scalar_tensor_tensor(
                            csl, csl, 1.0, tig[:], OP.mult, OP.add)
                        tc_ = tmp.tile([H, NN], bf16x, tag="tb", name="tc_")
                        nc.scalar.activation(tc_[:], csl, AF.Tanh)
                        to3 = to[:].rearrange(
                            "p (b w) -> p b w", w=WB)[:, :, lo:lo + 128]
                        tc3 = tc_[:].rearrange(
                            "p (b w) -> p b w", w=WB)[:, :, lo:lo + 128]
                        nc.vector.tensor_tensor(
                            i3(mslice(f * FREE), b0, n, 1), to3, tc3, OP.mult)
                    # head1: r = relu(conv(h,Wo1[:, :, :2]) + partial1)
                    for co in range(2):
                        ph = psum.tile([H, 3 * WB], f32, tag="ps", name="ps")
                        prs = [(P_H0 + c0 - 1, P_H0 + c0 + 1),
                               (P_H1 + c0 - 1, P_H1 + c0 + 1),
                               (P_H0 + c0, P_H1 + c0),
                               (P_P1(co) + c0, P_Z)]
                        bnds = [40 + co * 3, 40 + co * 3 + 1,
                                40 + co * 3 + 2, 52 + co]
                        for j, (oa, ob) in enumerate(prs):
                            nc.tensor.matmul(ph[:, :NN], pband(bnds[j]),
                                             pair_mv(smeg, oa, ob, NN),
                                             start=(j == 0), stop=(j == 3),
                                             perf_mode=DR)
                        hp3 = ph[:, :NN].rearrange(
                            "p (b w) -> p b w", w=WB)[:, :, lo:lo + 128]
                        nc.scalar.activation(
                            i3(s2meg[:, co * FREE:co * FREE + FREE], b0, n, 1),
                            hp3, AF.Relu)
                    head2_and_lp(xt, st, b0, n, c0, lo)

            # ---------------- deferred lp phase (poly-exp on DVE) ----------------
            # stored y = -2*(log_std+bo2[1]); lp += sum_w(0.5*d^2*e^y - y/2)
            # e^y via degree-4 polynomial (|y| < 0.75, rel err ~5e-4), all on
            # DVE so no activation-table switches; the scheduler freely
            # overlaps these with the step-loop matmuls.
            A0, A1, A2, A3, A4 = (1.0000054, 0.99936129, 0.49979731,
                                  0.17194260, 0.04274371)
            IW = BL * 128
            for st in range(16):
                sl = slice(st * IW, (st + 1) * IW)
                yv = dst_s[:, sl]
                t1 = tmp.tile([H, IW], bf16, tag="tq", name="t1", bufs=4)
                nc.vector.tensor_scalar(t1[:], yv, A4, None, OP.mult)
                t2 = tmp.tile([H, IW], bf16, tag="tq", name="t2", bufs=4)
                nc.vector.scalar_tensor_tensor(t2[:], t1[:], A3, yv,
                                               OP.add, OP.mult)
                t3 = tmp.tile([H, IW], bf16, tag="tq", name="t3", bufs=4)
                nc.vector.scalar_tensor_tensor(t3[:], t2[:], A2, yv,
                                               OP.add, OP.mult)
                t4 = tmp.tile([H, IW], bf16, tag="tq", name="t4", bufs=4)
                nc.vector.scalar_tensor_tensor(t4[:], t3[:], A1, yv,
                                               OP.add, OP.mult)
                q = tmp.tile([H, IW], bf16, tag="tq", name="q", bufs=4)
                nc.vector.scalar_tensor_tensor(
                    q[:], dst_d[:, sl], 0.5, dst_d[:, sl], OP.mult, OP.mult)
                t5 = tmp.tile([H, IW], bf16, tag="tq", name="t5", bufs=4)
                nc.vector.scalar_tensor_tensor(t5[:], t4[:], A0, q[:],
                                               OP.add, OP.mult)
                t6 = tmp.tile([H, IW], bf16, tag="tq", name="t6", bufs=4)
                nc.vector.scalar_tensor_tensor(t6[:], yv, -0.5, t5[:],
                                               OP.mult, OP.add)
                red = tmp.tile([H, BL], f32, tag="tr", name="red", bufs=3)
                nc.vector.reduce_sum(
                    red[:], t6[:].rearrange("p (b w) -> p b w", w=128), AX.X)
                nc.vector.tensor_tensor(lp[:], lp[:], red[:], OP.add)

            # final: out = -(sum_p lp) - 16*128*128*HALF_LOG_2PI
            po = psum.tile([BL, 1], f32, tag="ps", name="ps")
            nc.tensor.matmul(po[:], lp[:], ones[:], start=True, stop=True)
            osb = state.tile([BL, 1], f32, tag="osb", name="osb")
            nc.scalar.activation(osb[:], po[:], AF.Identity,
                                 scale=-1.0, bias=bap(16, BL))
            nc.sync.dma_start(od[:], osb[:])
    nc.compile()
    return nc


def kernel(**inputs):
    import ml_dtypes
    x = np.ascontiguousarray(inputs["x"], np.float32).astype(ml_dtypes.bfloat16)
    cond = np.ascontiguousarray(inputs["cond"], np.float32)
    bands = _build_bands(
        np.asarray(inputs["Wci"], np.float32),
        np.asarray(inputs["Wc1"], np.float32),
        np.asarray(inputs["Wc2"], np.float32),
        np.asarray(inputs["Wo1"], np.float32),
        np.asarray(inputs["Wo2"], np.float32),
        np.asarray(inputs["Wih"], np.float32),
        np.asarray(inputs["Whh"], np.float32))
    ubands = _build_bands(
        np.asarray(inputs["Wci"], np.float32),
        np.asarray(inputs["Wc1"], np.float32),
        np.asarray(inputs["Wc2"], np.float32),
        np.asarray(inputs["Wo1"], np.float32),
        np.asarray(inputs["Wo2"], np.float32),
        np.asarray(inputs["Wih"], np.float32),
        np.asarray(inputs["Whh"], np.float32))[
        N_ONETIME_BANDS:N_ONETIME_BANDS + 3].astype(ml_dtypes.bfloat16)
    pbands = _build_pair_bands(
        np.asarray(inputs["Wih"], np.float32),
        np.asarray(inputs["Whh"], np.float32),
        np.asarray(inputs["Wo1"], np.float32),
        np.asarray(inputs["Wo2"], np.float32))
    nc = _build_program(
        np.asarray(inputs["bci"], np.float32),
        np.asarray(inputs["bc1"], np.float32),
        np.asarray(inputs["bc2"], np.float32),
        np.asarray(inputs["bo1"], np.float32),
        np.asarray(inputs["bo2"], np.float32),
        np.asarray(inputs["bih"], np.float32))
    from concourse.bass_utils import run_bass_kernel_spmd
    in_maps = [
        {"x": x[i * BL:(i + 1) * BL], "cond": cond[i * BL:(i + 1) * BL],
         "bands": bands, "pbands": pbands, "ubands": ubands}
        for i in range(NCORES)
    ]
    res = run_bass_kernel_spmd(nc, in_maps, list(range(NCORES)))
    out = np.concatenate(
        [res.results[i]["out"].reshape(BL) for i in range(NCORES)])
    return out.astype(np.float32)


if __name__ == "__main__":
    # smoke test with tiny random weights
    rng = np.random.default_rng(0)
    ins = {
        "x": rng.standard_normal((64, 16, 128, 128), np.float32),
        "cond": rng.standard_normal((64, 16, 128, 128), np.float32),
        "Wci": rng.standard_normal((3, 3, 1, 1), np.float32) * 0.1,
        "bci": np.zeros(1, np.float32),
        "Wc1": rng.standard_normal((3, 3, 16, 2), np.float32) * 0.1,
        "bc1": np.zeros(2, np.float32),
        "Wc2": rng.standard_normal((3, 3, 2, 2), np.float32) * 0.1,
        "bc2": np.zeros(2, np.float32),
        "Wo1": rng.standard_normal((3, 3, 4, 2), np.float32) * 0.1,
        "bo1": np.zeros(2, np.float32),
        "Wo2": rng.standard_normal((3, 3, 2, 2), np.float32) * 0.1,
        "bo2": np.zeros(2, np.float32),
        "Wih": rng.standard_normal((3, 3, 1, 8), np.float32) * 0.1,
        "bih": np.zeros(8, np.float32),
        "Whh": rng.standard_normal((3, 3, 2, 8), np.float32) * 0.1,
    }
    print(kernel(**ins)[:8])

